# revision 43
# baseline (speedup 1.0000x reference)
"""Trainium2 Bass kernel: batched single-head self-attention.

Reference computation (per (b, l) pair, 20 independent blocks):
    X = x[b, l] viewed as [N=1024, D=256] (xf layout)
    out[b, l] = softmax(beta * X @ X.T, axis=-1) @ X

Device algorithm (per block):
  * Scores: S[m, n] = sum_d X^T[d, m] X^T[d, n] on the TensorEngine with
    D on partitions.  All matmul MOVING operands are bf16 (PE streams
    bf16 at 1 cyc/col; fp32r takes ~2).  bf16 everywhere costs ~7e-3
    rel-max error on this data vs the 2e-2 gate (fp64-oracle verified).
  * Softmax shift W = exp(beta*(S - c_n)): the per-query -c row arrives
    as a 12KB DRAM row, is replicated across partitions by GpSimd
    partition_broadcast, and is added to the PSUM scores by
    scalar_tensor_tensor ops ALTERNATING between VectorE and GpSimd
    (one engine alone cannot keep up with the PE).  ScalarE then exps
    the shifted fp32 tile straight to bf16 W and does nothing else.
  * Second matmul: O[n, d] = sum_m W[m, n] xfo[m, d] with the W slice
    [128, 128] STATIONARY and xfo[m, 0:258] = [x | 1 | 0] moving, so
    every streamed column feeds 128 output rows and Z_n falls out as
    output column 256.  No separate Z pass exists.
  * The work is organized as two h-SWEEPS per block (query halves):
    sweep h computes the 8 score tiles [128k x 512q] and the 4 O
    accumulators for its query half, O trailing scores by 3 key tiles.
    This maps the 8-bank PSUM exactly (4 score ring + 4 O accumulators)
    and means sweep 0 only needs the first halves of xb -- which drives
    the DMA plan below.
  * The three DMA queues (Sync/Activation/Pool) run at only ~45GB/s
    each, so input transfers are split into chunks issued in
    consumption order: xb slab0 quarters first (the 2 queues' heads),
    then the -c row, then xfo0 / later slabs.  Outputs are written as
    bf16 (halves the out traffic; +~2e-3 error) and each 2-tile
    evacuation pair is chased by its own DMA on a per-slab queue.

Host pre/post (layout + O(N*D) work only; all O(N^2*D) flops on device):
  * xb   = X^T in bf16                  (score operands)
  * xf   = [X | 1 | 0] in bf16          (value operand)
  * nb   = -||x_n||^2 as one fp32 row   (shift, replicated on device)
  * out  = O[:, :256] / O[:, 256:257]   (normalize; already [n, d])

Sharding: 20 blocks over 8 cores as 2 full blocks + 1 half block (512
queries) per core -- exact, no padded compute.  The half blocks use a
host-side rotation of the key axis so every core runs the identical
program (softmax is invariant to key permutation when values are
permuted identically).
"""

import numpy as np
import ml_dtypes

import concourse.tile as tile
from concourse import bacc, mybir
from concourse.bass_utils import run_bass_kernel_spmd

F32 = mybir.dt.float32
BF16 = mybir.dt.bfloat16

B, L, D, H, W = 4, 5, 256, 32, 32
N = H * W            # 1024 keys per block
NBLK = B * L         # 20
NCORES = 8
NFULL = 2            # full blocks per core
NSLAB = 3            # 2 full + 1 half
DF = 272             # value operand row: [x | 1 | 0 | pad...] -- padded so
                     # bf16 rows stay 32B-aligned (272*2 = 544 = 17*32)
DO = 258             # O matmul moving width / output row: [d0..d255, Z, 0]

EXP = mybir.ActivationFunctionType.Exp
ALU = mybir.AluOpType


def build_program(beta: float):
    nc = bacc.Bacc("TRN2", target_bir_lowering=False, debug=False,
                   num_devices=NCORES)
    # inputs are pre-chunked on the host so every DMA is a contiguous
    # [128, *] transfer (hardware DGE) of ~256-272KB: compute gates on
    # half-tensors, while descriptor-issue overhead (~650ns per
    # dma_start on the queue engine) stays amortized
    xb_in = nc.dram_tensor("xb_in", [NSLAB, 2, 128, 2, 512], BF16,
                           kind="ExternalInput")
    xf_in = nc.dram_tensor("xf_in", [NSLAB, 2, 128, 4, DF], BF16,
                           kind="ExternalInput")
    nb_in = nc.dram_tensor("nb_in", [1, NSLAB * N], BF16,
                           kind="ExternalInput")
    y_out = nc.dram_tensor("y_out", [NFULL, 128, 8, DO], BF16,
                           kind="ExternalOutput")
    y2a_out = nc.dram_tensor("y2a_out", [128, 2, DO], BF16,
                             kind="ExternalOutput")
    y2b_out = nc.dram_tensor("y2b_out", [128, 2, DO], BF16,
                             kind="ExternalOutput")

    with tile.TileContext(nc) as tc:
        _build(tc, nc, xb_in.ap(), xf_in.ap(), nb_in.ap(), y_out.ap(),
               y2a_out.ap(), y2b_out.ap(), beta)
    nc.finalize()
    return nc


def _build(tc, nc, xb_in, xf_in, nb_in, y_out, y2a_out, y2b_out, beta):
    import contextlib
    ctx = contextlib.ExitStack()
    with ctx:
        const = ctx.enter_context(tc.tile_pool(name="const", bufs=1))
        xb_pool = ctx.enter_context(tc.tile_pool(name="xb", bufs=NSLAB))
        xfo_pool = ctx.enter_context(tc.tile_pool(name="xfo", bufs=NSLAB))
        nb_pool = ctx.enter_context(tc.tile_pool(name="nb", bufs=1))
        ssh_pool = ctx.enter_context(tc.tile_pool(name="ssh", bufs=6))
        # W tiles stay live until the h1 sweep at the end of the block.
        w_pool = ctx.enter_context(tc.tile_pool(name="w", bufs=10))
        o_sb_pool = ctx.enter_context(tc.tile_pool(name="o_sb", bufs=3))
        # PSUM: 4-deep score ring + 4 O accumulators = 8 banks.
        ps_s = ctx.enter_context(tc.tile_pool(name="ps_s", bufs=4, space="PSUM"))
        ps_o = ctx.enter_context(tc.tile_pool(name="ps_o", bufs=4, space="PSUM"))

        # Warm the PE clock (HAM) during the input-DMA window --
        # otherwise the first ~4us of real matmuls run at reduced clock.
        warm_src = const.tile([128, 512], F32)
        nc.gpsimd.memset(warm_src[:], 0.0)
        ones_f32 = const.tile([1, 128], F32)
        nc.gpsimd.memset(ones_f32[:], 1.0)
        ones_bf = const.tile([1, 128], BF16)
        nc.vector.tensor_copy(ones_bf[:], ones_f32[:])
        for wi in range(3):
            warm_ps = ps_o.tile([128, 512], F32, tag="o", name=f"warm_{wi}")
            nc.tensor.matmul(warm_ps[:], warm_src[:, 0:128], warm_src[:],
                             start=True, stop=True)

        # ---- input DMA plan ---------------------------------------
        # The 16 DMA engines are shared by all queues and sustain only
        # ~100GB/s aggregate, so descriptor ISSUE order is consumption
        # order: the Sync queue (whose engine is otherwise idle) gets
        # the critical-path FIFO xb0, xfo0, xb1, xb2; the Activation
        # queue issues the 6KB shift row now and the later xfo / output
        # transfers from mid-kernel program positions.  All transfers
        # are whole-tensor contiguous (hardware DGE; strided slices
        # fall back to ~3x-slower software DGE).
        nb_row = nb_pool.tile([1, NSLAB * N], BF16, tag="nb_row")
        nc.scalar.dma_start(out=nb_row[:], in_=nb_in[:])
        xbs, xfos = [], []
        for s in range(NSLAB):
            # [128, n-half, c(d-half), 512]: n-half-major so each DMA
            # chunk is one contiguous [128, 2, 512] write
            xb = xb_pool.tile([128, 2, 2, 512], BF16, tag="xb",
                              name=f"xb_{s}")
            xbs.append(xb)
            # [128, a-half, a%4, DF]: each DMA chunk [128, 4, DF]
            xfo = xfo_pool.tile([128, 2, 4, DF], BF16, tag="xfo",
                                name=f"xfo_{s}")
            xfos.append(xfo)
        # single strict-FIFO queue, chunks in exact consumption order --
        # a second input queue would steal shared-DMA-engine bandwidth
        # from the critical first transfers (and the scheduler hoists
        # any dependency-free dma_start to t=0, so program position
        # cannot stagger them)
        for s in range(NSLAB):
            for nh in range(2):
                nc.sync.dma_start(out=xbs[s][:, nh, :, :],
                                  in_=xb_in[s][nh])
            for ah in range(2):
                nc.sync.dma_start(out=xfos[s][:, ah, :, :],
                                  in_=xf_in[s][ah])

        # -c replicated to 128 partitions: a K=1 PE matmul per 512-query
        # chunk (ones x row, ~0.2us) + one VectorE evacuation.  The
        # shift value itself needs no precision -- a per-query constant
        # cancels exactly in O/Z -- it only has to be within ~30 nats
        # of the row max, so bf16 inputs are fine.
        nb_all = nb_pool.tile([128, NSLAB * N], F32, tag="nb")

        def bcast_shift(s, h):
            cs = slice(s * N + h * 512, s * N + (h + 1) * 512)
            bc = ps_s.tile([128, 512], F32, tag="sps", name=f"bc_{s}_{h}")
            nc.tensor.matmul(bc[:], ones_bf[:], nb_row[:, cs],
                             start=True, stop=True)
            nc.vector.tensor_copy(nb_all[:, cs], bc[:])

        bcast_shift(0, 0)
        bcast_shift(0, 1)

        for s in range(NSLAB):
            n_q = N if s < NFULL else N // 2
            n_sw = n_q // 512   # h sweeps (1 or 2)
            xb, xfo = xbs[s], xfos[s]
            if s >= 1:
                # replicate the next slab's shift chunks now (cheap PE
                # matmul + DVE copy, off the critical path)
                for hh in range(n_sw):
                    bcast_shift(s, hh)

            wt_tiles = [w_pool.tile([128, N], BF16, tag="w",
                                    name=f"w_{s}_{a}") for a in range(8)]
            n_t = n_q // 128
            o_sb = o_sb_pool.tile([128, n_t, DO], BF16, tag="o_sb",
                                  name=f"o_sb_{s}")

            for h in range(n_sw):
                hs = slice(h * 512, (h + 1) * 512)
                o_tiles = {}

                def scores(a):
                    asl = slice((a % 4) * 128, (a % 4) * 128 + 128)
                    sp = ps_s.tile([128, 512], F32, tag="sps",
                                   name=f"sps_{s}_{h}_{a}")
                    for c in range(2):
                        nc.tensor.matmul(sp[:], xb[:, a // 4, c, asl],
                                         xb[:, h, c, :],
                                         start=(c == 0), stop=(c == 1))
                    # shift: s_sh = S + (-c_n) on the VectorE (GpSimd
                    # cannot read PSUM; with one tile per sweep step the
                    # VectorE keeps up with the PE on its own)
                    ssh = ssh_pool.tile([128, 512], F32, tag="ssh",
                                        name=f"ssh_{s}_{h}_{a}")
                    nc.vector.scalar_tensor_tensor(
                        ssh[:], sp[:], 1.0,
                        nb_all[:, s * N + h * 512: s * N + (h + 1) * 512],
                        ALU.mult, ALU.add)
                    # W = exp(beta * s_sh) -> bf16; ScalarE does only exp
                    nc.scalar.activation(wt_tiles[a][:, hs], ssh[:], EXP,
                                         scale=float(beta))

                def emit_o(a):
                    # O[q] += W[a][:, q].T @ xfo[a]; column 256 = Z
                    for q in range(4 * h, 4 * h + 4):
                        if a == 0:
                            o_tiles[q] = ps_o.tile([128, DO], F32, tag="o",
                                                   name=f"o_{s}_{h}_{q}")
                        qs = slice(q * 128, (q + 1) * 128)
                        nc.tensor.matmul(o_tiles[q][:], wt_tiles[a][:, qs],
                                         xfo[:, a // 4, a % 4, 0:DO],
                                         start=(a == 0), stop=(a == 7))

                # O trails the scores far enough that neither the
                # VectorE/ScalarE W chain nor the previous sweep's
                # evacuations gate it (the first sweep's O matmuls ride
                # the progressive xfo0 chunk arrivals)
                trail = 4
                for a in range(8):
                    scores(a)
                    if a >= trail:
                        emit_o(a - trail)
                for a in range(8 - trail, 8):
                    emit_o(a)
                # evacuate to bf16, split across DVE and ACT; the half
                # slab's pairs go straight out on their own compact DRAM
                # tensors so the tail overlaps copy and DMA
                for pi, p0 in enumerate(range(4 * h, 4 * h + 4, 2)):
                    nc.vector.tensor_copy(o_sb[:, p0, :], o_tiles[p0][:])
                    nc.scalar.copy(o_sb[:, p0 + 1, :], o_tiles[p0 + 1][:])
                    if s == NFULL:
                        nc.scalar.dma_start(
                            out=(y2a_out if pi == 0 else y2b_out)[:],
                            in_=o_sb[:, p0:p0 + 2, :])

            # one whole-tile (hardware-DGE) output DMA per full slab.
            # Slab 0's rides the slow-but-isolated Pool queue (it has
            # ~20us of slack and steals no bandwidth from the input
            # FIFO); slab 1's takes the fast Activation queue.
            if s == 0:
                nc.gpsimd.dma_start(out=y_out[s], in_=o_sb[:])
            elif s == 1:
                nc.scalar.dma_start(out=y_out[s], in_=o_sb[:])


_PROG_CACHE = {}


def _get_program(beta: float):
    if beta not in _PROG_CACHE:
        _PROG_CACHE[beta] = build_program(beta)
    return _PROG_CACHE[beta]


def make_in_maps(x: np.ndarray):
    """Shard the full input [B, L, D, H, W] into 8 per-core input maps."""
    xt_all = np.ascontiguousarray(x.reshape(NBLK, D, N))
    in_maps = []
    for c in range(NCORES):
        half_blk = NFULL * NCORES + c // 2
        half = xt_all[half_blk]
        if c % 2 == 1:
            # rotate keys so this core's queries are columns 0..511
            half = np.concatenate([half[:, N // 2:], half[:, :N // 2]], axis=1)
        slabs = np.stack([xt_all[NFULL * c], xt_all[NFULL * c + 1], half])
        xf = np.zeros((NSLAB, N, DF), np.float32)
        xf[:, :, :D] = slabs.transpose(0, 2, 1)
        xf[:, :, D] = 1.0
        negc = -np.einsum('sdn,sdn->sn', slabs, slabs)
        # pack into chunked device layout:
        #   xb [s, nh(query-half), 128, c(d-half), 512]
        #   xf [s, ah(a-half), 128, a%4, DF]
        xb_p = (slabs.reshape(NSLAB, 2, 128, 2, 512)
                .transpose(0, 3, 2, 1, 4))
        xf_p = (xf.reshape(NSLAB, 2, 4, 128, DF)
                .transpose(0, 1, 3, 2, 4))
        in_maps.append({
            "xb_in": np.ascontiguousarray(xb_p.astype(ml_dtypes.bfloat16)),
            "xf_in": np.ascontiguousarray(xf_p.astype(ml_dtypes.bfloat16)),
            "nb_in": np.ascontiguousarray(
                negc.reshape(1, NSLAB * N).astype(ml_dtypes.bfloat16)),
        })
    return in_maps


def assemble_output(results):
    """Normalize and gather per-core outputs into [B, L, N, D]."""
    out = np.empty((NBLK, N, D), np.float32)
    for c in range(NCORES):
        # y_out [NFULL, 128, 8, DO]: [q-within-tile, q-tile, feature]
        y = (results[c]["y_out"].astype(np.float32)
             .transpose(0, 2, 1, 3).reshape(NFULL, N, DO))
        y2 = (np.concatenate([results[c]["y2a_out"], results[c]["y2b_out"]],
                             axis=1).astype(np.float32)
              .transpose(1, 0, 2).reshape(N // 2, DO))
        for s, blk, lo, n_q in ((0, NFULL * c, 0, N),
                                (1, NFULL * c + 1, 0, N),
                                (2, NFULL * NCORES + c // 2,
                                 (c % 2) * (N // 2), N // 2)):
            o = y[s] if s < NFULL else y2
            out[blk, lo:lo + n_q] = o[:, :D] / o[:, D:D + 1]
    return out.reshape(B, L, N, D)


def kernel(x, beta, _trace=False, _fast=True):
    x = np.asarray(x, dtype=np.float32)
    assert x.shape == (B, L, D, H, W), x.shape
    beta_f = float(np.asarray(beta))
    prog = _get_program(beta_f)
    in_maps = make_in_maps(x)
    res = run_bass_kernel_spmd(prog, in_maps, core_ids=list(range(NCORES)),
                               trace=_trace)
    out = assemble_output(res.results)
    if _trace:
        return out, res
    return out


# revision 47
# speedup vs baseline: 1.1667x; 1.1667x over previous
"""Trainium2 Bass kernel: batched single-head self-attention.

Reference computation (per (b, l) pair, 20 independent blocks):
    X = x[b, l] viewed as [N=1024, D=256] (xf layout)
    out[b, l] = softmax(beta * X @ X.T, axis=-1) @ X

Device algorithm (per block):
  * Scores: S[m, n] = sum_d X^T[d, m] X^T[d, n] on the TensorEngine with
    D on partitions.  All matmul MOVING operands are bf16 (PE streams
    bf16 at 1 cyc/col; fp32r takes ~2).  bf16 everywhere costs ~7e-3
    rel-max error on this data vs the 2e-2 gate (fp64-oracle verified).
  * Softmax shift W = exp(beta*(S - c_n)): the per-query -c row arrives
    as a 12KB DRAM row, is replicated across partitions by GpSimd
    partition_broadcast, and is added to the PSUM scores by
    scalar_tensor_tensor ops ALTERNATING between VectorE and GpSimd
    (one engine alone cannot keep up with the PE).  ScalarE then exps
    the shifted fp32 tile straight to bf16 W and does nothing else.
  * Second matmul: O[n, d] = sum_m W[m, n] xfo[m, d] with the W slice
    [128, 128] STATIONARY and xfo[m, 0:258] = [x | 1 | 0] moving, so
    every streamed column feeds 128 output rows and Z_n falls out as
    output column 256.  No separate Z pass exists.
  * The work is organized as two h-SWEEPS per block (query halves):
    sweep h computes the 8 score tiles [128k x 512q] and the 4 O
    accumulators for its query half, O trailing scores by 3 key tiles.
    This maps the 8-bank PSUM exactly (4 score ring + 4 O accumulators)
    and means sweep 0 only needs the first halves of xb -- which drives
    the DMA plan below.
  * The three DMA queues (Sync/Activation/Pool) run at only ~45GB/s
    each, so input transfers are split into chunks issued in
    consumption order: xb slab0 quarters first (the 2 queues' heads),
    then the -c row, then xfo0 / later slabs.  Outputs are written as
    bf16 (halves the out traffic; +~2e-3 error) and each 2-tile
    evacuation pair is chased by its own DMA on a per-slab queue.

Host pre/post (layout + O(N*D) work only; all O(N^2*D) flops on device):
  * xb   = X^T in bf16                  (score operands)
  * xf   = [X | 1 | 0] in bf16          (value operand)
  * nb   = -||x_n||^2 as one fp32 row   (shift, replicated on device)
  * out  = O[:, :256] / O[:, 256:257]   (normalize; already [n, d])

Sharding: 20 blocks over 8 cores as 2 full blocks + 1 half block (512
queries) per core -- exact, no padded compute.  The half blocks use a
host-side rotation of the key axis so every core runs the identical
program (softmax is invariant to key permutation when values are
permuted identically).
"""

import numpy as np
import ml_dtypes

import concourse.tile as tile
from concourse import bacc, mybir
from concourse.bass_utils import run_bass_kernel_spmd

F32 = mybir.dt.float32
BF16 = mybir.dt.bfloat16

B, L, D, H, W = 4, 5, 256, 32, 32
N = H * W            # 1024 keys per block
NBLK = B * L         # 20
NCORES = 8
NFULL = 2            # full blocks per core
NSLAB = 3            # 2 full + 1 half
DF = 272             # value operand row: [x | 1 | 0 | pad...] -- padded so
                     # bf16 rows stay 32B-aligned (272*2 = 544 = 17*32)
DO = 258             # O matmul moving width / output row: [d0..d255, Z, 0]

EXP = mybir.ActivationFunctionType.Exp
ALU = mybir.AluOpType


def build_program(beta: float):
    nc = bacc.Bacc("TRN2", target_bir_lowering=False, debug=False,
                   num_devices=NCORES)
    xb_in = nc.dram_tensor("xb_in", [NSLAB, 128, 2, N], BF16,
                           kind="ExternalInput")
    xf_in = nc.dram_tensor("xf_in", [NSLAB, 128, 8, DF], BF16,
                           kind="ExternalInput")
    nb_in = nc.dram_tensor("nb_in", [1, NSLAB * N], BF16,
                           kind="ExternalInput")
    y_out = nc.dram_tensor("y_out", [NFULL, 128, 8, DO], BF16,
                           kind="ExternalOutput")
    y2a_out = nc.dram_tensor("y2a_out", [128, 2, DO], BF16,
                             kind="ExternalOutput")
    y2b_out = nc.dram_tensor("y2b_out", [128, 2, DO], BF16,
                             kind="ExternalOutput")

    with tile.TileContext(nc) as tc:
        _build(tc, nc, xb_in.ap(), xf_in.ap(), nb_in.ap(), y_out.ap(),
               y2a_out.ap(), y2b_out.ap(), beta)
    nc.finalize()
    return nc


def _build(tc, nc, xb_in, xf_in, nb_in, y_out, y2a_out, y2b_out, beta):
    import contextlib
    ctx = contextlib.ExitStack()
    with ctx:
        const = ctx.enter_context(tc.tile_pool(name="const", bufs=1))
        xb_pool = ctx.enter_context(tc.tile_pool(name="xb", bufs=NSLAB))
        xfo_pool = ctx.enter_context(tc.tile_pool(name="xfo", bufs=NSLAB))
        nb_pool = ctx.enter_context(tc.tile_pool(name="nb", bufs=1))
        ssh_pool = ctx.enter_context(tc.tile_pool(name="ssh", bufs=3))
        # W tiles stay live until the h1 sweep at the end of the block.
        w_pool = ctx.enter_context(tc.tile_pool(name="w", bufs=2))
        o_sb_pool = ctx.enter_context(tc.tile_pool(name="o_sb", bufs=3))
        # PSUM: 4-deep score ring + 4 O accumulators = 8 banks.
        ps_s = ctx.enter_context(tc.tile_pool(name="ps_s", bufs=4, space="PSUM"))
        ps_o = ctx.enter_context(tc.tile_pool(name="ps_o", bufs=4, space="PSUM"))

        # Warm the PE clock (HAM) during the input-DMA window --
        # otherwise the first ~4us of real matmuls run at reduced clock.
        warm_src = const.tile([128, 512], F32)
        nc.gpsimd.memset(warm_src[:], 0.0)
        ones_f32 = const.tile([1, 128], F32)
        nc.gpsimd.memset(ones_f32[:], 1.0)
        ones_bf = const.tile([1, 128], BF16)
        nc.vector.tensor_copy(ones_bf[:], ones_f32[:])
        for wi in range(3):
            warm_ps = ps_o.tile([128, 512], F32, tag="o", name=f"warm_{wi}")
            nc.tensor.matmul(warm_ps[:], warm_src[:, 0:128], warm_src[:],
                             start=True, stop=True)

        # ---- input DMA plan ---------------------------------------
        # The 16 DMA engines are shared by all queues and sustain only
        # ~100GB/s aggregate, so descriptor ISSUE order is consumption
        # order: the Sync queue (whose engine is otherwise idle) gets
        # the critical-path FIFO xb0, xfo0, xb1, xb2; the Activation
        # queue issues the 6KB shift row now and the later xfo / output
        # transfers from mid-kernel program positions.  All transfers
        # are whole-tensor contiguous (hardware DGE; strided slices
        # fall back to ~3x-slower software DGE).
        nb_row = nb_pool.tile([1, NSLAB * N], BF16, tag="nb_row")
        nc.scalar.dma_start(out=nb_row[:], in_=nb_in[:])
        xbs, xfos = [], []
        for s in range(NSLAB):
            xb = xb_pool.tile([128, 2, N], BF16, tag="xb", name=f"xb_{s}")
            xbs.append(xb)
            xfo = xfo_pool.tile([128, 8, DF], BF16, tag="xfo",
                                name=f"xfo_{s}")
            xfos.append(xfo)
        # single strict-FIFO queue in consumption order -- a second
        # input queue would steal shared-DMA-engine bandwidth from the
        # critical first transfers (the scheduler hoists any dependency-
        # free dma_start to t=0, so program position cannot stagger them)
        for s in range(NSLAB):
            nc.sync.dma_start(out=xbs[s][:], in_=xb_in[s])
            nc.sync.dma_start(out=xfos[s][:], in_=xf_in[s])

        # -c replicated to 128 partitions: a K=1 PE matmul per 512-query
        # chunk (ones x row, ~0.2us) + one VectorE evacuation.  The
        # shift value itself needs no precision -- a per-query constant
        # cancels exactly in O/Z -- it only has to be within ~30 nats
        # of the row max, so bf16 inputs are fine.
        nb_all = nb_pool.tile([128, NSLAB * N], F32, tag="nb")

        def bcast_shift(s, h):
            cs = slice(s * N + h * 512, s * N + (h + 1) * 512)
            bc = ps_s.tile([128, 512], F32, tag="sps", name=f"bc_{s}_{h}")
            nc.tensor.matmul(bc[:], ones_bf[:], nb_row[:, cs],
                             start=True, stop=True)
            nc.vector.tensor_copy(nb_all[:, cs], bc[:])

        bcast_shift(0, 0)
        bcast_shift(0, 1)

        for s in range(NSLAB):
            n_q = N if s < NFULL else N // 2
            n_sw = n_q // 512   # h sweeps (1 or 2)
            xb, xfo = xbs[s], xfos[s]
            if s >= 1:
                # replicate the next slab's shift chunks now (cheap PE
                # matmul + DVE copy, off the critical path)
                for hh in range(n_sw):
                    bcast_shift(s, hh)

            # one flat W tile per slab: [128, h-half, a*512 + col], so a
            # PAIRED exp can write two key tiles' worth in one ACT op
            w_all = w_pool.tile([128, 2, 8 * 512], BF16, tag="w",
                                name=f"w_{s}")
            n_t = n_q // 128
            o_sb = o_sb_pool.tile([128, n_t, DO], BF16, tag="o_sb",
                                  name=f"o_sb_{s}")

            for h in range(n_sw):
                hs = slice(h * 512, (h + 1) * 512)
                o_tiles = {}
                ssh_cur = [None]

                def scores(a):
                    asl = slice(a * 128, (a + 1) * 128)
                    sp = ps_s.tile([128, 512], F32, tag="sps",
                                   name=f"sps_{s}_{h}_{a}")
                    for c in range(2):
                        nc.tensor.matmul(sp[:], xb[:, c, asl],
                                         xb[:, c, hs],
                                         start=(c == 0), stop=(c == 1))
                    # shift: s_sh = S + (-c_n) on the VectorE (GpSimd
                    # cannot read PSUM; with one tile per sweep step the
                    # VectorE keeps up with the PE on its own)
                    if a % 2 == 0:
                        ssh_cur[0] = ssh_pool.tile([128, 1024], F32,
                                                   tag="ssh",
                                                   name=f"ssh_{s}_{h}_{a}")
                    ssh = ssh_cur[0]
                    nc.vector.scalar_tensor_tensor(
                        ssh[:, (a % 2) * 512:(a % 2) * 512 + 512], sp[:],
                        1.0,
                        nb_all[:, s * N + h * 512: s * N + (h + 1) * 512],
                        ALU.mult, ALU.add)
                    # W = exp(beta * s_sh) -> bf16, one PAIRED ACT op per
                    # two key tiles (halves ScalarE instruction overhead)
                    if a % 2 == 1:
                        nc.scalar.activation(
                            w_all[:, h, (a - 1) * 512:(a + 1) * 512],
                            ssh[:], EXP, scale=float(beta))

                def emit_o(a):
                    # O[q] += W[a][:, q].T @ xfo[a]; column 256 = Z
                    for q in range(4 * h, 4 * h + 4):
                        if a == 0:
                            o_tiles[q] = ps_o.tile([128, DO], F32, tag="o",
                                                   name=f"o_{s}_{h}_{q}")
                        ws = slice(a * 512 + (q % 4) * 128,
                                   a * 512 + (q % 4) * 128 + 128)
                        nc.tensor.matmul(o_tiles[q][:], w_all[:, h, ws],
                                         xfo[:, a, 0:DO],
                                         start=(a == 0), stop=(a == 7))

                # O trails the scores far enough that neither the
                # VectorE/ScalarE W chain (one pair-step deeper now) nor
                # the xfo0 arrival nor evacuations gate it
                trail = 6
                for a in range(8):
                    scores(a)
                    if a >= trail:
                        emit_o(a - trail)
                for a in range(8 - trail, 8):
                    emit_o(a)
                # evacuate to bf16, split across DVE and ACT; the half
                # slab's pairs go straight out on their own compact DRAM
                # tensors so the tail overlaps copy and DMA
                for pi, p0 in enumerate(range(4 * h, 4 * h + 4, 2)):
                    nc.vector.tensor_copy(o_sb[:, p0, :], o_tiles[p0][:])
                    nc.scalar.copy(o_sb[:, p0 + 1, :], o_tiles[p0 + 1][:])
                    if s == NFULL:
                        nc.scalar.dma_start(
                            out=(y2a_out if pi == 0 else y2b_out)[:],
                            in_=o_sb[:, p0:p0 + 2, :])

            # one whole-tile (hardware-DGE) output DMA per full slab,
            # on the fast Activation queue (it only carries the 6KB
            # shift row early; the Pool queue runs ~3x slower)
            if s < NFULL:
                nc.scalar.dma_start(out=y_out[s], in_=o_sb[:])


_PROG_CACHE = {}


def _get_program(beta: float):
    if beta not in _PROG_CACHE:
        _PROG_CACHE[beta] = build_program(beta)
    return _PROG_CACHE[beta]


def make_in_maps(x: np.ndarray):
    """Shard the full input [B, L, D, H, W] into 8 per-core input maps."""
    xt_all = np.ascontiguousarray(x.reshape(NBLK, D, N))
    in_maps = []
    for c in range(NCORES):
        half_blk = NFULL * NCORES + c // 2
        half = xt_all[half_blk]
        if c % 2 == 1:
            # rotate keys so this core's queries are columns 0..511
            half = np.concatenate([half[:, N // 2:], half[:, :N // 2]], axis=1)
        slabs = np.stack([xt_all[NFULL * c], xt_all[NFULL * c + 1], half])
        xf = np.zeros((NSLAB, N, DF), np.float32)
        xf[:, :, :D] = slabs.transpose(0, 2, 1)
        xf[:, :, D] = 1.0
        negc = -np.einsum('sdn,sdn->sn', slabs, slabs)
        # pack into device layout: xb [128, 2, N], xf [128, 8, DF]
        xb_p = slabs.reshape(NSLAB, 2, 128, N).transpose(0, 2, 1, 3)
        xf_p = xf.reshape(NSLAB, 8, 128, DF).transpose(0, 2, 1, 3)
        in_maps.append({
            "xb_in": np.ascontiguousarray(xb_p.astype(ml_dtypes.bfloat16)),
            "xf_in": np.ascontiguousarray(xf_p.astype(ml_dtypes.bfloat16)),
            "nb_in": np.ascontiguousarray(
                negc.reshape(1, NSLAB * N).astype(ml_dtypes.bfloat16)),
        })
    return in_maps


def assemble_output(results):
    """Normalize and gather per-core outputs into [B, L, N, D]."""
    out = np.empty((NBLK, N, D), np.float32)
    for c in range(NCORES):
        # y_out [NFULL, 128, 8, DO]: [q-within-tile, q-tile, feature]
        y = (results[c]["y_out"].astype(np.float32)
             .transpose(0, 2, 1, 3).reshape(NFULL, N, DO))
        y2 = (np.concatenate([results[c]["y2a_out"], results[c]["y2b_out"]],
                             axis=1).astype(np.float32)
              .transpose(1, 0, 2).reshape(N // 2, DO))
        for s, blk, lo, n_q in ((0, NFULL * c, 0, N),
                                (1, NFULL * c + 1, 0, N),
                                (2, NFULL * NCORES + c // 2,
                                 (c % 2) * (N // 2), N // 2)):
            o = y[s] if s < NFULL else y2
            out[blk, lo:lo + n_q] = o[:, :D] / o[:, D:D + 1]
    return out.reshape(B, L, N, D)


def kernel(x, beta, _trace=False, _fast=True):
    x = np.asarray(x, dtype=np.float32)
    assert x.shape == (B, L, D, H, W), x.shape
    beta_f = float(np.asarray(beta))
    prog = _get_program(beta_f)
    in_maps = make_in_maps(x)
    res = run_bass_kernel_spmd(prog, in_maps, core_ids=list(range(NCORES)),
                               trace=_trace)
    out = assemble_output(res.results)
    if _trace:
        return out, res
    return out
